# revision 12
# baseline (speedup 1.0000x reference)
"""Fused multi-head attention (B=4, L=2048, D=1024, H=16) on 8 NeuronCores.

Sharding: core c handles batch b=c//2 and query rows [1024*(c%2), +1024).
Per-core input x is the batch's [2048, 1024] activations ROTATED so the
core's own query rows are rows 0..1023 (softmax over keys is permutation
invariant). No collectives.

V2 design (vs baseline): phases A (projections) and B (attention) are FUSED.
The ScalarE exp stream (~294us of ACT work) is the hard floor of phase B;
projection matmuls for head-pair p+1 are interleaved into the PE gaps of
pair p's attention units so PE and ACT run concurrently. All SBUF operands
are bf16 (halves SBUF + DMA; PE rate is unchanged; PSUM accumulation stays
fp32). V lives entirely in SBUF (no DRAM staging round-trip). bv is folded
into an effective output bias bo' = Wo^T bv + bo on the host.

Per-unit pipeline (unit = (pair, lh, st), lookahead-1):
  scores^T tile [s,2,l]: per head-sub matmul K_h^T@Q_h (64-contraction,
  auto row-tiled T0/T8 via base partitions); exp via ScalarE -> bf16 e2;
  1-2 interleaved projection matmuls for pair p+1; PV accumulate
  [V_h|1]^T @ e2 -> po [65, 2, 512] PSUM (row 64 = denominator).
Epilogue per (pair, lh): copy po, reciprocal of row 64, partition
broadcast, multiply -> ot (bf16).
Phase C: y^T = Wo^T @ OT (+bo' fused via ScalarE), PE-transpose back,
DMA out from PSUM directly.
"""

import numpy as np

import sys

for _p in ("/opt/trn_rl_repo", "/opt/pypackages"):
    if _p not in sys.path:
        sys.path.append(_p)

from contextlib import ExitStack

import concourse.bass as bass
import concourse.mybir as mybir
import concourse.tile as tile
from concourse import bacc
from concourse.bass_utils import run_bass_kernel_spmd
from concourse.masks import make_identity

B, L, D, H = 4, 2048, 1024, 16
HD = D // H  # 64
LQ = 1024  # query rows per core
N_CORES = 8
F32 = mybir.dt.float32
F32R = mybir.dt.float32r
BF16 = mybir.dt.bfloat16
AF = mybir.ActivationFunctionType

P = 128
KT_TILES = D // P  # 8
ST_TILES = L // P  # 16
DT_TILES = D // P  # 8
LH = 512
SCALE = 1.0 / float(np.sqrt(HD))
NPAIR = H // 2  # 8

TR_BF16 = True  # bf16 x + bf16 PE transposes (else fp32 x, f32r transposes)
X_DT = BF16 if TR_BF16 else F32


def _load_bias(nc, pool, dram, name):
    t = pool.tile([P, DT_TILES], F32, name=name)
    nc.gpsimd.dma_start(t[:], dram.rearrange("(t p) -> p t", p=P))
    return t


def build_nc(repeat=1, stop_after=None):
    nc = bacc.Bacc(None)

    x_d = nc.declare_dram_parameter("x", [L, D], X_DT, isOutput=False)
    wq_d = nc.declare_dram_parameter("wq", [D, D], BF16, isOutput=False)
    wk_d = nc.declare_dram_parameter("wk", [D, D], BF16, isOutput=False)
    wv_d = nc.declare_dram_parameter("wv", [D, D], BF16, isOutput=False)
    wo_d = nc.declare_dram_parameter("wo", [D, D], BF16, isOutput=False)
    bq_d = nc.declare_dram_parameter("bq", [D], F32, isOutput=False)
    bk_d = nc.declare_dram_parameter("bk", [D], F32, isOutput=False)
    bo_d = nc.declare_dram_parameter("bo", [D], F32, isOutput=False)
    y_d = nc.declare_dram_parameter("y", [LQ, D], BF16, isOutput=True)

    with tile.TileContext(nc) as tc, ExitStack() as ctx:
      for _rep in range(repeat):
       with ExitStack() as rctx:
        singles = rctx.enter_context(tc.tile_pool(name="singles", bufs=1))
        ident32 = singles.tile([P, P], F32, name="ident32")
        make_identity(nc, ident32[:])
        identb = singles.tile([P, P], BF16 if TR_BF16 else F32R, name="identb")
        nc.vector.tensor_copy(identb[:], ident32[:])
        bq_sb = _load_bias(nc, singles, bq_d, "bq")
        bk_sb = _load_bias(nc, singles, bk_d, "bk")
        bo_sb = _load_bias(nc, singles, bo_d, "bo")

        ot_pool = rctx.enter_context(tc.tile_pool(name="ot", bufs=1))
        ot = ot_pool.tile([P, DT_TILES, LQ], BF16, name="ot")

        with ExitStack() as fctx:
            xt_pool = fctx.enter_context(tc.tile_pool(name="xt", bufs=1))
            qt_pool = fctx.enter_context(tc.tile_pool(name="qt", bufs=1))
            kt_pool = fctx.enter_context(tc.tile_pool(name="kt", bufs=1))
            vs_pool = fctx.enter_context(tc.tile_pool(name="vs", bufs=1))
            wpool = fctx.enter_context(tc.tile_pool(name="wpool", bufs=2))
            e2_pool = fctx.enter_context(tc.tile_pool(name="e2", bufs=3))
            rr_pool = fctx.enter_context(tc.tile_pool(name="rr", bufs=2))
            rb_pool = fctx.enter_context(tc.tile_pool(name="rb", bufs=2))
            otmp_pool = fctx.enter_context(tc.tile_pool(name="otmp", bufs=2))
            ps_proj = fctx.enter_context(
                tc.tile_pool(name="ps_proj", bufs=2, space="PSUM")
            )
            ps_s_pool = fctx.enter_context(
                tc.tile_pool(name="ps_s", bufs=2, space="PSUM")
            )
            po_pool = fctx.enter_context(
                tc.tile_pool(name="po", bufs=1, space="PSUM")
            )

            xt = xt_pool.tile([P, KT_TILES, L], BF16, name="xt")
            qt = qt_pool.tile([P, DT_TILES, LQ], BF16, name="qt")
            kt = kt_pool.tile([P, DT_TILES, L], BF16, name="kt")
            vsb = vs_pool.tile([P, ST_TILES, H, HD + 1], BF16, name="vsb")
            nc.vector.memset(vsb[:, :, :, HD : HD + 1], 1.0)

            # ---------- A-slice machinery ----------
            def make_qk_slices(p):
                """Closures emitting pair p's Wq/Wk DMAs + Q/K matmuls/evicts."""
                ws = {}
                fns = []

                def dma_w(p=p):
                    for nm, wd in (("q", wq_d), ("k", wk_d)):
                        t = wpool.tile([P, KT_TILES, P], BF16, name=f"w{nm}")
                        nc.gpsimd.dma_start(
                            t[:],
                            wd[:, p * P : (p + 1) * P].rearrange(
                                "(t p) n -> p t n", p=P
                            ),
                        )
                        ws[nm] = t

                fns.append(dma_w)

                for w_key, b_sb, out_sb, ncols in (
                    ("q", bq_sb, qt, LQ),
                    ("k", bk_sb, kt, L),
                ):
                    for ci in range(ncols // LH):
                        for kg in range(4):

                            def mm(w_key=w_key, b_sb=b_sb, out_sb=out_sb,
                                   ci=ci, kg=kg, p=p):
                                if kg == 0:
                                    ws[("ps", w_key, ci)] = ps_proj.tile(
                                        [P, LH], F32, name="ps_qk", tag="ps"
                                    )
                                ps = ws[("ps", w_key, ci)]
                                for ki in (2 * kg, 2 * kg + 1):
                                    nc.tensor.matmul(
                                        ps[:],
                                        ws[w_key][:, ki, :],
                                        xt[:, ki, ci * LH : (ci + 1) * LH],
                                        start=(ki == 0),
                                        stop=(ki == KT_TILES - 1),
                                    )
                                if kg == 3:
                                    nc.vector.tensor_scalar_add(
                                        out_sb[:, p, ci * LH : (ci + 1) * LH],
                                        ps[:],
                                        b_sb[:, p : p + 1],
                                    )

                            fns.append(mm)
                return fns

            def make_v_slices(dc):
                """V projection for head-pair block dc (8 heads, 512 cols)."""
                ws = {}
                fns = []

                def dma_w(dc=dc):
                    t = wpool.tile([P, KT_TILES, LH], BF16, name="wv")
                    nc.gpsimd.dma_start(
                        t[:],
                        wv_d[:, dc * LH : (dc + 1) * LH].rearrange(
                            "(t p) n -> p t n", p=P
                        ),
                    )
                    ws["v"] = t

                fns.append(dma_w)

                for stv in range(ST_TILES):
                    for kg in range(4):

                        def mm(stv=stv, kg=kg, dc=dc):
                            if kg == 0:
                                ws[("ps", stv)] = ps_proj.tile(
                                    [P, 8, HD], F32, name="ps_v", tag="ps"
                                )
                            ps = ws[("ps", stv)]
                            for ki in (2 * kg, 2 * kg + 1):
                                nc.tensor.matmul(
                                    ps[:],
                                    xt[:, ki, stv * P : (stv + 1) * P],
                                    ws["v"][:, ki, :],
                                    start=(ki == 0),
                                    stop=(ki == KT_TILES - 1),
                                )
                            if kg == 3:
                                nc.vector.tensor_copy(
                                    vsb[:, stv, dc * 8 : (dc + 1) * 8, 0:HD],
                                    ps[:],
                                )

                        fns.append(mm)
                return fns

            # ---------- startup: pair-0 weights, x transpose, A(0) ----------
            qk0 = make_qk_slices(0)
            v0 = make_v_slices(0)
            qk0[0]()  # W DMAs first (overlap with x load)
            v0[0]()

            def transpose_li(li, xpool):
                x_sb = xpool.tile([P, D], X_DT, name="x_sb")
                nc.sync.dma_start(x_sb[:], x_d[li * P : (li + 1) * P, :])
                for kg in range(KT_TILES // 4):
                    pt4 = ps_proj.tile(
                        [P, 4, P], BF16 if TR_BF16 else F32, name="pt4", tag="ps"
                    )
                    for b in range(4):
                        ki = 4 * kg + b
                        nc.tensor.transpose(
                            pt4[:, b, :],
                            x_sb[:, ki * P : (ki + 1) * P],
                            identb[:] if TR_BF16 else ident32[:],
                        )
                    nc.vector.tensor_copy(
                        xt[:, 4 * kg : 4 * kg + 4, li * P : (li + 1) * P],
                        pt4[:],
                    )

            xpool = fctx.enter_context(tc.tile_pool(name="xp", bufs=2))

            def transpose_slices(li):
                st8 = {}
                fns = []

                def dma_x(li=li):
                    t = xpool.tile([P, D], X_DT, name="x_sb")
                    nc.sync.dma_start(t[:], x_d[li * P : (li + 1) * P, :])
                    st8["x"] = t

                fns.append(dma_x)
                for kg in range(KT_TILES // 4):

                    def tr(kg=kg, li=li):
                        pt4 = ps_proj.tile(
                            [P, 4, P], BF16 if TR_BF16 else F32, name="pt4",
                            tag="ps",
                        )
                        for b in range(4):
                            ki = 4 * kg + b
                            nc.tensor.transpose(
                                pt4[:, b, :],
                                st8["x"][:, ki * P : (ki + 1) * P],
                                identb[:] if TR_BF16 else ident32[:],
                            )
                        nc.vector.tensor_copy(
                            xt[:, 4 * kg : 4 * kg + 4, li * P : (li + 1) * P],
                            pt4[:],
                        )

                    fns.append(tr)
                return fns

            # startup: only what unit 0 strictly needs runs serially; the
            # rest slides into B(0)'s PE gaps bounded by per-unit deadlines
            feed = []  # (deadline_global_unit, fn), sorted by deadline
            for li in range(ST_TILES):
                for fn in transpose_slices(li):
                    feed.append((max(0, 4 * (li // 4) - 2), fn))
            for fn in qk0[1:5]:
                feed.append((0, fn))
            for fn in qk0[5:9]:
                feed.append((ST_TILES, fn))
            for ci in range(4):
                for fn in qk0[9 + 4 * ci : 13 + 4 * ci]:
                    feed.append((4 * ci, fn))
            for stv in range(ST_TILES):
                for fn in v0[1 + 4 * stv : 5 + 4 * stv]:
                    feed.append((stv + 1, fn))
            feed.sort(key=lambda t: t[0])

            # ---------- fused B(p) + A(p+1) ----------
            def scores_exp(p, lh, st):
                ps_s = ps_s_pool.tile([P, 2, LH], F32, name="ps_s")
                for sub in range(2):
                    nc.tensor.matmul(
                        ps_s[:, sub, :],
                        kt[sub * HD : (sub + 1) * HD, p, st * P : (st + 1) * P],
                        qt[sub * HD : (sub + 1) * HD, p, lh * LH : (lh + 1) * LH],
                        start=True,
                        stop=True,
                    )
                e2 = e2_pool.tile([P, 2, LH], BF16, name="e2")
                nc.scalar.activation(e2[:], ps_s[:], AF.Exp, scale=SCALE)
                return e2

            def pv(p, lh, st, e2, po):
                for sub in range(2):
                    nc.tensor.matmul(
                        po[:, sub, :],
                        vsb[:, st, 2 * p + sub, :],
                        e2[:, sub, :],
                        start=(st == 0),
                        stop=(st == ST_TILES - 1),
                    )

            def epilogue(p, lh, po):
                o_tmp = otmp_pool.tile([HD + 1, 2, LH], F32, name="o_tmp")
                nc.vector.tensor_copy(o_tmp[:], po[:])
                for sub in range(2):
                    r = rr_pool.tile([1, LH], F32, name="r_row")
                    nc.vector.reciprocal(r[:], o_tmp[HD : HD + 1, sub, :])
                    rb = rb_pool.tile([HD, LH], F32, name="r_bc")
                    nc.gpsimd.partition_broadcast(rb[:], r[:])
                    nc.gpsimd.tensor_mul(
                        ot[sub * HD : (sub + 1) * HD, p, lh * LH : (lh + 1) * LH],
                        o_tmp[0:HD, sub, :],
                        rb[:],
                    )

            units = [
                (p, lh, st)
                for p in range(NPAIR)
                for lh in range(2)
                for st in range(ST_TILES)
            ]
            po_map = {}
            prev = None
            for g, (p, lh, st) in enumerate(units):
                if lh == 0 and st == 0:
                    if p + 1 < NPAIR:
                        for fn in make_qk_slices(p + 1):
                            feed.append((2 * ST_TILES * (p + 1), fn))
                    if p == 0:
                        for fn in make_v_slices(1):
                            feed.append((4 * 2 * ST_TILES, fn))
                    feed.sort(key=lambda t: t[0])
                while feed and feed[0][0] <= g:
                    feed.pop(0)[1]()
                e2 = scores_exp(p, lh, st)
                if feed:
                    d = feed[0][0]
                    units_until = max(1, d - g - 2)
                    if not (st >= ST_TILES - 2 and units_until > 8):
                        n_due = sum(1 for dl, _ in feed if dl == d)
                        k = min(3, len(feed), max(1, -(-n_due // units_until)))
                        for _ in range(k):
                            feed.pop(0)[1]()
                if prev is not None:
                    pp, plh, pst, pe2 = prev
                    if (pp, plh) not in po_map:
                        po_map[(pp, plh)] = po_pool.tile(
                            [HD + 1, 2, LH], F32, name="po"
                        )
                    pv(pp, plh, pst, pe2, po_map[(pp, plh)])
                    if pst == ST_TILES - 1:
                        epilogue(pp, plh, po_map.pop((pp, plh)))
                prev = (p, lh, st, e2)
            pp, plh, pst, pe2 = prev
            if (pp, plh) not in po_map:
                po_map[(pp, plh)] = po_pool.tile([HD + 1, 2, LH], F32, name="po")
            pv(pp, plh, pst, pe2, po_map[(pp, plh)])
            epilogue(pp, plh, po_map.pop((pp, plh)))

            if stop_after == "ab":
                tmp = otmp_pool.tile([P, LQ], BF16, name="dbg2")
                nc.vector.tensor_copy(tmp[:], ot[:, 7, :])
                nc.sync.dma_start(y_d[0:P, :], tmp[:])

        if stop_after == "ab":
            continue

        # ---------- C: output projection + transpose ----------
        with (
            tc.tile_pool(name="wo", bufs=2) as wo_pool,
            tc.tile_pool(name="gt", bufs=2) as gt_pool,
            tc.tile_pool(name="yb", bufs=3) as yb_pool,
            tc.tile_pool(name="ps_g", bufs=2, space="PSUM") as ps_g_pool,
            tc.tile_pool(name="ps_t", bufs=3, space="PSUM") as ps_t_pool,
        ):
            for j in range(DT_TILES):
                wo_sb = wo_pool.tile([P, KT_TILES, P], BF16, name="wo_sb")
                nc.gpsimd.dma_start(
                    wo_sb[:],
                    wo_d[:, j * P : (j + 1) * P].rearrange("(t p) n -> p t n", p=P),
                )
                gt_s = gt_pool.tile([P, LQ], BF16, name="gt_s")
                for lh in range(2):
                    ps_g = ps_g_pool.tile([P, LH], F32, name="ps_g")
                    for ki in range(KT_TILES):
                        nc.tensor.matmul(
                            ps_g[:],
                            wo_sb[:, ki, :],
                            ot[:, ki, lh * LH : (lh + 1) * LH],
                            start=(ki == 0),
                            stop=(ki == KT_TILES - 1),
                        )
                    nc.scalar.activation(
                        gt_s[:, lh * LH : (lh + 1) * LH],
                        ps_g[:],
                        AF.Identity,
                        bias=bo_sb[:, j : j + 1],
                    )
                for a in range(KT_TILES // 4):
                    pt4 = ps_t_pool.tile([P, 4, P], BF16, name="pt4_out")
                    for b in range(4):
                        i = 4 * a + b
                        nc.tensor.transpose(
                            pt4[:, b, :], gt_s[:, i * P : (i + 1) * P], identb[:]
                        )
                    yb = yb_pool.tile([P, 4, P], BF16, name="yb")
                    nc.vector.tensor_copy(yb[:], pt4[:])
                    nc.sync.dma_start(
                        y_d[4 * a * P : (4 * a + 4) * P, j * P : (j + 1) * P]
                        .rearrange("(b p) n -> p b n", p=P),
                        yb[:],
                    )

    nc.finalize()
    return nc


def _np_bf16():
    import ml_dtypes

    return ml_dtypes.bfloat16


def make_in_maps(inputs):
    """Host-side prep: rotate/shard x, cast to bf16, fold bv into bo."""
    bf16 = _np_bf16()
    q = np.asarray(inputs["q"], dtype=np.float32)
    Wq = np.asarray(inputs["Wq"], dtype=np.float32)
    Wk = np.asarray(inputs["Wk"], dtype=np.float32)
    Wv = np.asarray(inputs["Wv"], dtype=np.float32)
    Wo = np.asarray(inputs["Wo"], dtype=np.float32)
    bq = np.asarray(inputs["bq"], dtype=np.float32)
    bk = np.asarray(inputs["bk"], dtype=np.float32)
    bv = np.asarray(inputs["bv"], dtype=np.float32)
    bo = np.asarray(inputs["bo"], dtype=np.float32)

    bo_eff = (bv @ Wo + bo).astype(np.float32)
    x_dt = bf16 if TR_BF16 else np.float32
    wq_b = np.ascontiguousarray(Wq.astype(bf16))
    wk_b = np.ascontiguousarray(Wk.astype(bf16))
    wv_b = np.ascontiguousarray(Wv.astype(bf16))
    wo_b = np.ascontiguousarray(Wo.astype(bf16))

    in_maps = []
    for c in range(N_CORES):
        b, half = c // 2, c % 2
        lo = LQ * half
        x_rot = np.concatenate([q[b, lo:], q[b, :lo]], axis=0).astype(x_dt)
        in_maps.append({
            "x": np.ascontiguousarray(x_rot),
            "wq": wq_b, "wk": wk_b, "wv": wv_b, "wo": wo_b,
            "bq": bq, "bk": bk, "bo": bo_eff,
        })
    return in_maps


_NC_CACHE = None


def kernel(**inputs):
    global _NC_CACHE
    if _NC_CACHE is None:
        _NC_CACHE = build_nc()
    nc = _NC_CACHE

    in_maps = make_in_maps(inputs)
    res = run_bass_kernel_spmd(nc, in_maps, core_ids=list(range(N_CORES)))

    out = np.empty((B, L, D), dtype=np.float32)
    for c in range(N_CORES):
        b, half = c // 2, c % 2
        lo = LQ * half
        out[b, lo : lo + LQ, :] = np.asarray(res.results[c]["y"]).astype(np.float32)
    return out


# revision 13
# speedup vs baseline: 1.0642x; 1.0642x over previous
"""Fused multi-head attention (B=4, L=2048, D=1024, H=16) on 8 NeuronCores.

Sharding: core c handles batch b=c//2 and query rows [1024*(c%2), +1024).
Per-core input x is the batch's [2048, 1024] activations ROTATED so the
core's own query rows are rows 0..1023 (softmax over keys is permutation
invariant). No collectives.

V2 design (vs baseline): phases A (projections) and B (attention) are FUSED.
The ScalarE exp stream (~294us of ACT work) is the hard floor of phase B;
projection matmuls for head-pair p+1 are interleaved into the PE gaps of
pair p's attention units so PE and ACT run concurrently. All SBUF operands
are bf16 (halves SBUF + DMA; PE rate is unchanged; PSUM accumulation stays
fp32). V lives entirely in SBUF (no DRAM staging round-trip). bv is folded
into an effective output bias bo' = Wo^T bv + bo on the host.

Per-unit pipeline (unit = (pair, lh, st), lookahead-1):
  scores^T tile [s,2,l]: per head-sub matmul K_h^T@Q_h (64-contraction,
  auto row-tiled T0/T8 via base partitions); exp via ScalarE -> bf16 e2;
  1-2 interleaved projection matmuls for pair p+1; PV accumulate
  [V_h|1]^T @ e2 -> po [65, 2, 512] PSUM (row 64 = denominator).
Epilogue per (pair, lh): copy po, reciprocal of row 64, partition
broadcast, multiply -> ot (bf16).
Phase C: y^T = Wo^T @ OT (+bo' fused via ScalarE), PE-transpose back,
DMA out from PSUM directly.
"""

import numpy as np

import sys

for _p in ("/opt/trn_rl_repo", "/opt/pypackages"):
    if _p not in sys.path:
        sys.path.append(_p)

from contextlib import ExitStack

import concourse.bass as bass
import concourse.mybir as mybir
import concourse.tile as tile
from concourse import bacc
from concourse.bass_utils import run_bass_kernel_spmd
from concourse.masks import make_identity

B, L, D, H = 4, 2048, 1024, 16
HD = D // H  # 64
LQ = 1024  # query rows per core
N_CORES = 8
F32 = mybir.dt.float32
F32R = mybir.dt.float32r
BF16 = mybir.dt.bfloat16
AF = mybir.ActivationFunctionType

P = 128
KT_TILES = D // P  # 8
ST_TILES = L // P  # 16
DT_TILES = D // P  # 8
LH = 512
SCALE = 1.0 / float(np.sqrt(HD))
NPAIR = H // 2  # 8

TR_BF16 = True  # bf16 x + bf16 PE transposes (else fp32 x, f32r transposes)
X_DT = BF16 if TR_BF16 else F32


def _load_bias(nc, pool, dram, name):
    t = pool.tile([P, DT_TILES], F32, name=name)
    nc.gpsimd.dma_start(t[:], dram.rearrange("(t p) -> p t", p=P))
    return t


def build_nc(repeat=1, stop_after=None):
    nc = bacc.Bacc(None)

    x_d = nc.declare_dram_parameter("x", [L, D], X_DT, isOutput=False)
    wq_d = nc.declare_dram_parameter("wq", [D, D], BF16, isOutput=False)
    wk_d = nc.declare_dram_parameter("wk", [D, D], BF16, isOutput=False)
    wv_d = nc.declare_dram_parameter("wv", [D, D], BF16, isOutput=False)
    wo_d = nc.declare_dram_parameter("wo", [D, D], BF16, isOutput=False)
    bq_d = nc.declare_dram_parameter("bq", [D], F32, isOutput=False)
    bk_d = nc.declare_dram_parameter("bk", [D], F32, isOutput=False)
    bo_d = nc.declare_dram_parameter("bo", [D], F32, isOutput=False)
    y_d = nc.declare_dram_parameter("y", [LQ, D], BF16, isOutput=True)

    with tile.TileContext(nc) as tc, ExitStack() as ctx:
      for _rep in range(repeat):
       with ExitStack() as rctx:
        singles = rctx.enter_context(tc.tile_pool(name="singles", bufs=1))
        ident32 = singles.tile([P, P], F32, name="ident32")
        make_identity(nc, ident32[:])
        identb = singles.tile([P, P], BF16 if TR_BF16 else F32R, name="identb")
        nc.vector.tensor_copy(identb[:], ident32[:])
        bq_sb = _load_bias(nc, singles, bq_d, "bq")
        bk_sb = _load_bias(nc, singles, bk_d, "bk")
        bo_sb = _load_bias(nc, singles, bo_d, "bo")

        ot_pool = rctx.enter_context(tc.tile_pool(name="ot", bufs=1))
        ot = ot_pool.tile([P, DT_TILES, LQ], BF16, name="ot")

        with ExitStack() as fctx:
            xt_pool = fctx.enter_context(tc.tile_pool(name="xt", bufs=1))
            qt_pool = fctx.enter_context(tc.tile_pool(name="qt", bufs=1))
            kt_pool = fctx.enter_context(tc.tile_pool(name="kt", bufs=1))
            vs_pool = fctx.enter_context(tc.tile_pool(name="vs", bufs=1))
            wpool = fctx.enter_context(tc.tile_pool(name="wpool", bufs=2))
            e2_pool = fctx.enter_context(tc.tile_pool(name="e2", bufs=3))
            rr_pool = fctx.enter_context(tc.tile_pool(name="rr", bufs=2))
            rb_pool = fctx.enter_context(tc.tile_pool(name="rb", bufs=2))
            otmp_pool = fctx.enter_context(tc.tile_pool(name="otmp", bufs=2))
            ps_proj = fctx.enter_context(
                tc.tile_pool(name="ps_proj", bufs=2, space="PSUM")
            )
            ps_s_pool = fctx.enter_context(
                tc.tile_pool(name="ps_s", bufs=2, space="PSUM")
            )
            po_pool = fctx.enter_context(
                tc.tile_pool(name="po", bufs=1, space="PSUM")
            )

            xt = xt_pool.tile([P, KT_TILES, L], BF16, name="xt")
            qt = qt_pool.tile([P, DT_TILES, LQ], BF16, name="qt")
            kt = kt_pool.tile([P, DT_TILES, L], BF16, name="kt")
            vsb = vs_pool.tile([P, ST_TILES, H, HD + 1], BF16, name="vsb")
            nc.vector.memset(vsb[:, :, :, HD : HD + 1], 1.0)

            # ---------- A-slice machinery ----------
            def make_qk_slices(p):
                """Closures emitting pair p's Wq/Wk DMAs + Q/K matmuls/evicts."""
                ws = {}
                fns = []

                def dma_w(p=p):
                    for nm, wd in (("q", wq_d), ("k", wk_d)):
                        t = wpool.tile([P, KT_TILES, P], BF16, name=f"w{nm}")
                        nc.gpsimd.dma_start(
                            t[:],
                            wd[:, p * P : (p + 1) * P].rearrange(
                                "(t p) n -> p t n", p=P
                            ),
                        )
                        ws[nm] = t

                fns.append(dma_w)

                for w_key, b_sb, out_sb, ncols in (
                    ("q", bq_sb, qt, LQ),
                    ("k", bk_sb, kt, L),
                ):
                    for ci in range(ncols // LH):
                        for kg in range(4):

                            def mm(w_key=w_key, b_sb=b_sb, out_sb=out_sb,
                                   ci=ci, kg=kg, p=p):
                                if kg == 0:
                                    ws[("ps", w_key, ci)] = ps_proj.tile(
                                        [P, LH], F32, name="ps_qk", tag="ps"
                                    )
                                ps = ws[("ps", w_key, ci)]
                                for ki in (2 * kg, 2 * kg + 1):
                                    nc.tensor.matmul(
                                        ps[:],
                                        ws[w_key][:, ki, :],
                                        xt[:, ki, ci * LH : (ci + 1) * LH],
                                        start=(ki == 0),
                                        stop=(ki == KT_TILES - 1),
                                    )
                                if kg == 3:
                                    nc.vector.tensor_scalar_add(
                                        out_sb[:, p, ci * LH : (ci + 1) * LH],
                                        ps[:],
                                        b_sb[:, p : p + 1],
                                    )

                            fns.append(mm)
                return fns

            def make_v_slices(dc):
                """V projection for head-pair block dc (8 heads, 512 cols)."""
                ws = {}
                fns = []

                def dma_w(dc=dc):
                    t = wpool.tile([P, KT_TILES, LH], BF16, name="wv")
                    nc.gpsimd.dma_start(
                        t[:],
                        wv_d[:, dc * LH : (dc + 1) * LH].rearrange(
                            "(t p) n -> p t n", p=P
                        ),
                    )
                    ws["v"] = t

                fns.append(dma_w)

                for stv in range(ST_TILES):
                    for kg in range(4):

                        def mm(stv=stv, kg=kg, dc=dc):
                            if kg == 0:
                                ws[("ps", stv)] = ps_proj.tile(
                                    [P, 8, HD], F32, name="ps_v", tag="ps"
                                )
                            ps = ws[("ps", stv)]
                            for ki in (2 * kg, 2 * kg + 1):
                                nc.tensor.matmul(
                                    ps[:],
                                    xt[:, ki, stv * P : (stv + 1) * P],
                                    ws["v"][:, ki, :],
                                    start=(ki == 0),
                                    stop=(ki == KT_TILES - 1),
                                )
                            if kg == 3:
                                nc.vector.tensor_copy(
                                    vsb[:, stv, dc * 8 : (dc + 1) * 8, 0:HD],
                                    ps[:],
                                )

                        fns.append(mm)
                return fns

            # ---------- startup: pair-0 weights, x transpose, A(0) ----------
            qk0 = make_qk_slices(0)
            v0 = make_v_slices(0)
            qk0[0]()  # W DMAs first (overlap with x load)
            v0[0]()

            def transpose_li(li, xpool):
                x_sb = xpool.tile([P, D], X_DT, name="x_sb")
                nc.sync.dma_start(x_sb[:], x_d[li * P : (li + 1) * P, :])
                for kg in range(KT_TILES // 4):
                    pt4 = ps_proj.tile(
                        [P, 4, P], BF16 if TR_BF16 else F32, name="pt4", tag="ps"
                    )
                    for b in range(4):
                        ki = 4 * kg + b
                        nc.tensor.transpose(
                            pt4[:, b, :],
                            x_sb[:, ki * P : (ki + 1) * P],
                            identb[:] if TR_BF16 else ident32[:],
                        )
                    nc.vector.tensor_copy(
                        xt[:, 4 * kg : 4 * kg + 4, li * P : (li + 1) * P],
                        pt4[:],
                    )

            with tc.tile_pool(name="xp", bufs=2) as xpool:
                for li in range(KT_TILES):
                    transpose_li(li, xpool)
                for fn in qk0[1:9]:  # Q(0) needs only l-tiles 0..7
                    fn()
                for li in range(KT_TILES, ST_TILES):
                    transpose_li(li, xpool)
            for fn in qk0[9:]:
                fn()
            for fn in v0[1:]:
                fn()


            # ---------- fused B(p) + A(p+1) ----------
            def scores_exp(p, lh, st):
                ps_s = ps_s_pool.tile([P, 2, LH], F32, name="ps_s")
                for sub in range(2):
                    nc.tensor.matmul(
                        ps_s[:, sub, :],
                        kt[sub * HD : (sub + 1) * HD, p, st * P : (st + 1) * P],
                        qt[sub * HD : (sub + 1) * HD, p, lh * LH : (lh + 1) * LH],
                        start=True,
                        stop=True,
                    )
                e2 = e2_pool.tile([P, 2, LH], BF16, name="e2")
                nc.scalar.activation(e2[:], ps_s[:], AF.Exp, scale=SCALE)
                return e2

            def pv(p, lh, st, e2, po):
                for sub in range(2):
                    nc.tensor.matmul(
                        po[:, sub, :],
                        vsb[:, st, 2 * p + sub, :],
                        e2[:, sub, :],
                        start=(st == 0),
                        stop=(st == ST_TILES - 1),
                    )

            def epilogue(p, lh, po):
                o_tmp = otmp_pool.tile([HD + 1, 2, LH], F32, name="o_tmp")
                nc.vector.tensor_copy(o_tmp[:], po[:])
                for sub in range(2):
                    r = rr_pool.tile([1, LH], F32, name="r_row")
                    nc.vector.reciprocal(r[:], o_tmp[HD : HD + 1, sub, :])
                    rb = rb_pool.tile([HD, LH], F32, name="r_bc")
                    nc.gpsimd.partition_broadcast(rb[:], r[:])
                    nc.vector.tensor_mul(
                        ot[sub * HD : (sub + 1) * HD, p, lh * LH : (lh + 1) * LH],
                        o_tmp[0:HD, sub, :],
                        rb[:],
                    )

            units = [
                (p, lh, st)
                for p in range(NPAIR)
                for lh in range(2)
                for st in range(ST_TILES)
            ]
            po_map = {}
            feed = []  # (deadline_pair, fn), kept sorted by deadline
            prev = None
            for g, (p, lh, st) in enumerate(units):
                if lh == 0 and st == 0:
                    if p + 1 < NPAIR:
                        for fn in make_qk_slices(p + 1):
                            feed.append((p + 1, fn))
                    if p == 0:
                        for fn in make_v_slices(1):
                            feed.append((4, fn))
                    feed.sort(key=lambda t: t[0])
                    while feed and feed[0][0] <= p:
                        feed.pop(0)[1]()
                e2 = scores_exp(p, lh, st)
                if feed:
                    d = feed[0][0]
                    units_until = max(1, d * 2 * ST_TILES - g - 4)
                    if not (st >= ST_TILES - 2 and units_until > 8):
                        n_due = sum(1 for dl, _ in feed if dl == d)
                        k = min(3, len(feed), max(1, -(-n_due // units_until)))
                        for _ in range(k):
                            feed.pop(0)[1]()
                if prev is not None:
                    pp, plh, pst, pe2 = prev
                    if (pp, plh) not in po_map:
                        po_map[(pp, plh)] = po_pool.tile(
                            [HD + 1, 2, LH], F32, name="po"
                        )
                    pv(pp, plh, pst, pe2, po_map[(pp, plh)])
                    if pst == ST_TILES - 1:
                        epilogue(pp, plh, po_map.pop((pp, plh)))
                prev = (p, lh, st, e2)
            pp, plh, pst, pe2 = prev
            if (pp, plh) not in po_map:
                po_map[(pp, plh)] = po_pool.tile([HD + 1, 2, LH], F32, name="po")
            pv(pp, plh, pst, pe2, po_map[(pp, plh)])
            epilogue(pp, plh, po_map.pop((pp, plh)))

            if stop_after == "ab":
                tmp = otmp_pool.tile([P, LQ], BF16, name="dbg2")
                nc.vector.tensor_copy(tmp[:], ot[:, 7, :])
                nc.sync.dma_start(y_d[0:P, :], tmp[:])

        if stop_after == "ab":
            continue

        # ---------- C: output projection + transpose ----------
        with (
            tc.tile_pool(name="wo", bufs=2) as wo_pool,
            tc.tile_pool(name="gt", bufs=2) as gt_pool,
            tc.tile_pool(name="yb", bufs=3) as yb_pool,
            tc.tile_pool(name="ps_g", bufs=2, space="PSUM") as ps_g_pool,
            tc.tile_pool(name="ps_t", bufs=3, space="PSUM") as ps_t_pool,
        ):
            for j in range(DT_TILES):
                wo_sb = wo_pool.tile([P, KT_TILES, P], BF16, name="wo_sb")
                nc.gpsimd.dma_start(
                    wo_sb[:],
                    wo_d[:, j * P : (j + 1) * P].rearrange("(t p) n -> p t n", p=P),
                )
                gt_s = gt_pool.tile([P, LQ], BF16, name="gt_s")
                for lh in range(2):
                    ps_g = ps_g_pool.tile([P, LH], F32, name="ps_g")
                    for ki in range(KT_TILES):
                        nc.tensor.matmul(
                            ps_g[:],
                            wo_sb[:, ki, :],
                            ot[:, ki, lh * LH : (lh + 1) * LH],
                            start=(ki == 0),
                            stop=(ki == KT_TILES - 1),
                        )
                    nc.scalar.activation(
                        gt_s[:, lh * LH : (lh + 1) * LH],
                        ps_g[:],
                        AF.Identity,
                        bias=bo_sb[:, j : j + 1],
                    )
                for a in range(KT_TILES // 4):
                    pt4 = ps_t_pool.tile([P, 4, P], BF16, name="pt4_out")
                    for b in range(4):
                        i = 4 * a + b
                        nc.tensor.transpose(
                            pt4[:, b, :], gt_s[:, i * P : (i + 1) * P], identb[:]
                        )
                    yb = yb_pool.tile([P, 4, P], BF16, name="yb")
                    nc.vector.tensor_copy(yb[:], pt4[:])
                    nc.sync.dma_start(
                        y_d[4 * a * P : (4 * a + 4) * P, j * P : (j + 1) * P]
                        .rearrange("(b p) n -> p b n", p=P),
                        yb[:],
                    )

    nc.finalize()
    return nc


def _np_bf16():
    import ml_dtypes

    return ml_dtypes.bfloat16


def make_in_maps(inputs):
    """Host-side prep: rotate/shard x, cast to bf16, fold bv into bo."""
    bf16 = _np_bf16()
    q = np.asarray(inputs["q"], dtype=np.float32)
    Wq = np.asarray(inputs["Wq"], dtype=np.float32)
    Wk = np.asarray(inputs["Wk"], dtype=np.float32)
    Wv = np.asarray(inputs["Wv"], dtype=np.float32)
    Wo = np.asarray(inputs["Wo"], dtype=np.float32)
    bq = np.asarray(inputs["bq"], dtype=np.float32)
    bk = np.asarray(inputs["bk"], dtype=np.float32)
    bv = np.asarray(inputs["bv"], dtype=np.float32)
    bo = np.asarray(inputs["bo"], dtype=np.float32)

    bo_eff = (bv @ Wo + bo).astype(np.float32)
    x_dt = bf16 if TR_BF16 else np.float32
    wq_b = np.ascontiguousarray(Wq.astype(bf16))
    wk_b = np.ascontiguousarray(Wk.astype(bf16))
    wv_b = np.ascontiguousarray(Wv.astype(bf16))
    wo_b = np.ascontiguousarray(Wo.astype(bf16))

    in_maps = []
    for c in range(N_CORES):
        b, half = c // 2, c % 2
        lo = LQ * half
        x_rot = np.concatenate([q[b, lo:], q[b, :lo]], axis=0).astype(x_dt)
        in_maps.append({
            "x": np.ascontiguousarray(x_rot),
            "wq": wq_b, "wk": wk_b, "wv": wv_b, "wo": wo_b,
            "bq": bq, "bk": bk, "bo": bo_eff,
        })
    return in_maps


_NC_CACHE = None


def kernel(**inputs):
    global _NC_CACHE
    if _NC_CACHE is None:
        _NC_CACHE = build_nc()
    nc = _NC_CACHE

    in_maps = make_in_maps(inputs)
    res = run_bass_kernel_spmd(nc, in_maps, core_ids=list(range(N_CORES)))

    out = np.empty((B, L, D), dtype=np.float32)
    for c in range(N_CORES):
        b, half = c // 2, c % 2
        lo = LQ * half
        out[b, lo : lo + LQ, :] = np.asarray(res.results[c]["y"]).astype(np.float32)
    return out


# revision 14
# speedup vs baseline: 8.3761x; 7.8707x over previous
"""Fused multi-head attention (B=4, L=2048, D=1024, H=16) on 8 NeuronCores.

Sharding: core c handles batch b=c//2 and query rows [1024*(c%2), +1024).
Per-core input x is the batch's [2048, 1024] activations ROTATED so the
core's own query rows are rows 0..1023 (softmax over keys is permutation
invariant). No collectives.

V2 design (vs baseline): phases A (projections) and B (attention) are FUSED.
The ScalarE exp stream (~294us of ACT work) is the hard floor of phase B;
projection matmuls for head-pair p+1 are interleaved into the PE gaps of
pair p's attention units so PE and ACT run concurrently. All SBUF operands
are bf16 (halves SBUF + DMA; PE rate is unchanged; PSUM accumulation stays
fp32). V lives entirely in SBUF (no DRAM staging round-trip). bv is folded
into an effective output bias bo' = Wo^T bv + bo on the host.

Per-unit pipeline (unit = (pair, lh, st), lookahead-1):
  scores^T tile [s,2,l]: per head-sub matmul K_h^T@Q_h (64-contraction,
  auto row-tiled T0/T8 via base partitions); exp via ScalarE -> bf16 e2;
  1-2 interleaved projection matmuls for pair p+1; PV accumulate
  [V_h|1]^T @ e2 -> po [65, 2, 512] PSUM (row 64 = denominator).
Epilogue per (pair, lh): copy po, reciprocal of row 64, partition
broadcast, multiply -> ot (bf16).
Phase C: y^T = Wo^T @ OT (+bo' fused via ScalarE), PE-transpose back,
DMA out from PSUM directly.
"""

import numpy as np

import sys

for _p in ("/opt/trn_rl_repo", "/opt/pypackages"):
    if _p not in sys.path:
        sys.path.append(_p)

from contextlib import ExitStack

import concourse.bass as bass
import concourse.mybir as mybir
import concourse.tile as tile
from concourse import bacc
from concourse.bass_utils import run_bass_kernel_spmd
from concourse.masks import make_identity

B, L, D, H = 4, 2048, 1024, 16
HD = D // H  # 64
LQ = 1024  # query rows per core
N_CORES = 8
F32 = mybir.dt.float32
F32R = mybir.dt.float32r
BF16 = mybir.dt.bfloat16
AF = mybir.ActivationFunctionType

P = 128
KT_TILES = D // P  # 8
ST_TILES = L // P  # 16
DT_TILES = D // P  # 8
LH = 512
SCALE = 1.0 / float(np.sqrt(HD))
NPAIR = H // 2  # 8

TR_BF16 = True  # bf16 x + bf16 PE transposes (else fp32 x, f32r transposes)
X_DT = BF16 if TR_BF16 else F32


def _load_bias(nc, pool, dram, name):
    t = pool.tile([P, DT_TILES], F32, name=name)
    nc.gpsimd.dma_start(t[:], dram.rearrange("(t p) -> p t", p=P))
    return t


def build_nc(repeat=1, stop_after=None):
    nc = bacc.Bacc(None)

    x_d = nc.declare_dram_parameter("x", [L, D], X_DT, isOutput=False)
    wq_d = nc.declare_dram_parameter("wq", [D, D], BF16, isOutput=False)
    wk_d = nc.declare_dram_parameter("wk", [D, D], BF16, isOutput=False)
    wv_d = nc.declare_dram_parameter("wv", [D, D], BF16, isOutput=False)
    wo_d = nc.declare_dram_parameter("wo", [D, D], BF16, isOutput=False)
    bq_d = nc.declare_dram_parameter("bq", [D], F32, isOutput=False)
    bk_d = nc.declare_dram_parameter("bk", [D], F32, isOutput=False)
    bo_d = nc.declare_dram_parameter("bo", [D], F32, isOutput=False)
    y_d = nc.declare_dram_parameter("y", [LQ, D], F32, isOutput=True)

    with tile.TileContext(nc) as tc, ExitStack() as ctx:
      for _rep in range(repeat):
       with ExitStack() as rctx:
        singles = rctx.enter_context(tc.tile_pool(name="singles", bufs=1))
        ident32 = singles.tile([P, P], F32, name="ident32")
        make_identity(nc, ident32[:])
        identb = singles.tile([P, P], BF16 if TR_BF16 else F32R, name="identb")
        nc.vector.tensor_copy(identb[:], ident32[:])
        identr = singles.tile([P, P], F32R, name="identr")
        nc.vector.tensor_copy(identr[:], ident32[:])
        bq_sb = _load_bias(nc, singles, bq_d, "bq")
        bk_sb = _load_bias(nc, singles, bk_d, "bk")
        bo_sb = _load_bias(nc, singles, bo_d, "bo")

        ot_pool = rctx.enter_context(tc.tile_pool(name="ot", bufs=1))
        ot = ot_pool.tile([P, DT_TILES, LQ], BF16, name="ot")

        with ExitStack() as fctx:
            xt_pool = fctx.enter_context(tc.tile_pool(name="xt", bufs=1))
            qt_pool = fctx.enter_context(tc.tile_pool(name="qt", bufs=1))
            kt_pool = fctx.enter_context(tc.tile_pool(name="kt", bufs=1))
            vs_pool = fctx.enter_context(tc.tile_pool(name="vs", bufs=1))
            wpool = fctx.enter_context(tc.tile_pool(name="wpool", bufs=2))
            e2_pool = fctx.enter_context(tc.tile_pool(name="e2", bufs=3))
            rr_pool = fctx.enter_context(tc.tile_pool(name="rr", bufs=2))
            rb_pool = fctx.enter_context(tc.tile_pool(name="rb", bufs=2))
            otmp_pool = fctx.enter_context(tc.tile_pool(name="otmp", bufs=2))
            ps_proj = fctx.enter_context(
                tc.tile_pool(name="ps_proj", bufs=2, space="PSUM")
            )
            ps_s_pool = fctx.enter_context(
                tc.tile_pool(name="ps_s", bufs=2, space="PSUM")
            )
            po_pool = fctx.enter_context(
                tc.tile_pool(name="po", bufs=1, space="PSUM")
            )

            xt = xt_pool.tile([P, KT_TILES, L], BF16, name="xt")
            qt = qt_pool.tile([P, DT_TILES, LQ], BF16, name="qt")
            kt = kt_pool.tile([P, DT_TILES, L], BF16, name="kt")
            vsb = vs_pool.tile([P, ST_TILES, H, HD + 1], BF16, name="vsb")
            nc.vector.memset(vsb[:, :, :, HD : HD + 1], 1.0)

            # ---------- A-slice machinery ----------
            def make_qk_slices(p):
                """Closures emitting pair p's Wq/Wk DMAs + Q/K matmuls/evicts."""
                ws = {}
                fns = []

                def dma_w(p=p):
                    for nm, wd in (("q", wq_d), ("k", wk_d)):
                        t = wpool.tile([P, KT_TILES, P], BF16, name=f"w{nm}")
                        nc.gpsimd.dma_start(
                            t[:],
                            wd[:, p * P : (p + 1) * P].rearrange(
                                "(t p) n -> p t n", p=P
                            ),
                        )
                        ws[nm] = t

                fns.append(dma_w)

                for w_key, b_sb, out_sb, ncols in (
                    ("q", bq_sb, qt, LQ),
                    ("k", bk_sb, kt, L),
                ):
                    for ci in range(ncols // LH):
                        for kg in range(4):

                            def mm(w_key=w_key, b_sb=b_sb, out_sb=out_sb,
                                   ci=ci, kg=kg, p=p):
                                if kg == 0:
                                    ws[("ps", w_key, ci)] = ps_proj.tile(
                                        [P, LH], F32, name="ps_qk", tag="ps"
                                    )
                                ps = ws[("ps", w_key, ci)]
                                for ki in (2 * kg, 2 * kg + 1):
                                    nc.tensor.matmul(
                                        ps[:],
                                        ws[w_key][:, ki, :],
                                        xt[:, ki, ci * LH : (ci + 1) * LH],
                                        start=(ki == 0),
                                        stop=(ki == KT_TILES - 1),
                                    )
                                if kg == 3:
                                    nc.vector.tensor_scalar_add(
                                        out_sb[:, p, ci * LH : (ci + 1) * LH],
                                        ps[:],
                                        b_sb[:, p : p + 1],
                                    )

                            fns.append(mm)
                return fns

            def make_v_slices(dc):
                """V projection for head-pair block dc (8 heads, 512 cols)."""
                ws = {}
                fns = []

                def dma_w(dc=dc):
                    t = wpool.tile([P, KT_TILES, LH], BF16, name="wv")
                    nc.gpsimd.dma_start(
                        t[:],
                        wv_d[:, dc * LH : (dc + 1) * LH].rearrange(
                            "(t p) n -> p t n", p=P
                        ),
                    )
                    ws["v"] = t

                fns.append(dma_w)

                for stv in range(ST_TILES):
                    for kg in range(4):

                        def mm(stv=stv, kg=kg, dc=dc):
                            if kg == 0:
                                ws[("ps", stv)] = ps_proj.tile(
                                    [P, 8, HD], F32, name="ps_v", tag="ps"
                                )
                            ps = ws[("ps", stv)]
                            for ki in (2 * kg, 2 * kg + 1):
                                nc.tensor.matmul(
                                    ps[:],
                                    xt[:, ki, stv * P : (stv + 1) * P],
                                    ws["v"][:, ki, :],
                                    start=(ki == 0),
                                    stop=(ki == KT_TILES - 1),
                                )
                            if kg == 3:
                                nc.vector.tensor_copy(
                                    vsb[:, stv, dc * 8 : (dc + 1) * 8, 0:HD],
                                    ps[:],
                                )

                        fns.append(mm)
                return fns

            # ---------- startup: pair-0 weights, x transpose, A(0) ----------
            qk0 = make_qk_slices(0)
            v0 = make_v_slices(0)
            qk0[0]()  # W DMAs first (overlap with x load)
            v0[0]()

            def transpose_li(li, xpool):
                x_sb = xpool.tile([P, D], X_DT, name="x_sb")
                nc.sync.dma_start(x_sb[:], x_d[li * P : (li + 1) * P, :])
                for kg in range(KT_TILES // 4):
                    pt4 = ps_proj.tile(
                        [P, 4, P], BF16 if TR_BF16 else F32, name="pt4", tag="ps"
                    )
                    for b in range(4):
                        ki = 4 * kg + b
                        nc.tensor.transpose(
                            pt4[:, b, :],
                            x_sb[:, ki * P : (ki + 1) * P],
                            identb[:] if TR_BF16 else ident32[:],
                        )
                    nc.vector.tensor_copy(
                        xt[:, 4 * kg : 4 * kg + 4, li * P : (li + 1) * P],
                        pt4[:],
                    )

            with tc.tile_pool(name="xp", bufs=2) as xpool:
                for li in range(KT_TILES):
                    transpose_li(li, xpool)
                for fn in qk0[1:9]:  # Q(0) needs only l-tiles 0..7
                    fn()
                for li in range(KT_TILES, ST_TILES):
                    transpose_li(li, xpool)
            for fn in qk0[9:]:
                fn()
            for fn in v0[1:]:
                fn()


            # ---------- fused B(p) + A(p+1) ----------
            def scores_exp(p, lh, st):
                ps_s = ps_s_pool.tile([P, 2, LH], F32, name="ps_s")
                for sub in range(2):
                    nc.tensor.matmul(
                        ps_s[:, sub, :],
                        kt[sub * HD : (sub + 1) * HD, p, st * P : (st + 1) * P],
                        qt[sub * HD : (sub + 1) * HD, p, lh * LH : (lh + 1) * LH],
                        start=True,
                        stop=True,
                    )
                e2 = e2_pool.tile([P, 2, LH], BF16, name="e2")
                nc.scalar.activation(e2[:], ps_s[:], AF.Exp, scale=SCALE)
                return e2

            def pv(p, lh, st, e2, po):
                for sub in range(2):
                    nc.tensor.matmul(
                        po[:, sub, :],
                        vsb[:, st, 2 * p + sub, :],
                        e2[:, sub, :],
                        start=(st == 0),
                        stop=(st == ST_TILES - 1),
                    )

            def epilogue(p, lh, po):
                o_tmp = otmp_pool.tile([HD + 1, 2, LH], F32, name="o_tmp")
                nc.vector.tensor_copy(o_tmp[:], po[:])
                for sub in range(2):
                    r = rr_pool.tile([1, LH], F32, name="r_row")
                    nc.vector.reciprocal(r[:], o_tmp[HD : HD + 1, sub, :])
                    rb = rb_pool.tile([HD, LH], F32, name="r_bc")
                    nc.gpsimd.partition_broadcast(rb[:], r[:])
                    nc.vector.tensor_mul(
                        ot[sub * HD : (sub + 1) * HD, p, lh * LH : (lh + 1) * LH],
                        o_tmp[0:HD, sub, :],
                        rb[:],
                    )

            units = [
                (p, lh, st)
                for p in range(NPAIR)
                for lh in range(2)
                for st in range(ST_TILES)
            ]
            po_map = {}
            feed = []  # (deadline_pair, fn), kept sorted by deadline
            prev = None
            for g, (p, lh, st) in enumerate(units):
                if lh == 0 and st == 0:
                    if p + 1 < NPAIR:
                        for fn in make_qk_slices(p + 1):
                            feed.append((p + 1, fn))
                    if p == 0:
                        for fn in make_v_slices(1):
                            feed.append((4, fn))
                    feed.sort(key=lambda t: t[0])
                    while feed and feed[0][0] <= p:
                        feed.pop(0)[1]()
                e2 = scores_exp(p, lh, st)
                if feed:
                    d = feed[0][0]
                    units_until = max(1, d * 2 * ST_TILES - g - 4)
                    if not (st >= ST_TILES - 2 and units_until > 8):
                        n_due = sum(1 for dl, _ in feed if dl == d)
                        k = min(3, len(feed), max(1, -(-n_due // units_until)))
                        for _ in range(k):
                            feed.pop(0)[1]()
                if prev is not None:
                    pp, plh, pst, pe2 = prev
                    if (pp, plh) not in po_map:
                        po_map[(pp, plh)] = po_pool.tile(
                            [HD + 1, 2, LH], F32, name="po"
                        )
                    pv(pp, plh, pst, pe2, po_map[(pp, plh)])
                    if pst == ST_TILES - 1:
                        epilogue(pp, plh, po_map.pop((pp, plh)))
                prev = (p, lh, st, e2)
            pp, plh, pst, pe2 = prev
            if (pp, plh) not in po_map:
                po_map[(pp, plh)] = po_pool.tile([HD + 1, 2, LH], F32, name="po")
            pv(pp, plh, pst, pe2, po_map[(pp, plh)])
            epilogue(pp, plh, po_map.pop((pp, plh)))

            if stop_after == "ab":
                tmp = otmp_pool.tile([P, LQ], F32, name="dbg2")
                nc.vector.tensor_copy(tmp[:], ot[:, 7, :])
                nc.sync.dma_start(y_d[0:P, :], tmp[:])

        if stop_after == "ab":
            continue

        # ---------- C: output projection + transpose ----------
        with (
            tc.tile_pool(name="wo", bufs=2) as wo_pool,
            tc.tile_pool(name="gt", bufs=2) as gt_pool,
            tc.tile_pool(name="yb", bufs=3) as yb_pool,
            tc.tile_pool(name="ps_g", bufs=2, space="PSUM") as ps_g_pool,
            tc.tile_pool(name="ps_t", bufs=3, space="PSUM") as ps_t_pool,
        ):
            for j in range(DT_TILES):
                wo_sb = wo_pool.tile([P, KT_TILES, P], BF16, name="wo_sb")
                nc.gpsimd.dma_start(
                    wo_sb[:],
                    wo_d[:, j * P : (j + 1) * P].rearrange("(t p) n -> p t n", p=P),
                )
                gt_s = gt_pool.tile([P, LQ], F32R, name="gt_s")
                for lh in range(2):
                    ps_g = ps_g_pool.tile([P, LH], F32, name="ps_g")
                    for ki in range(KT_TILES):
                        nc.tensor.matmul(
                            ps_g[:],
                            wo_sb[:, ki, :],
                            ot[:, ki, lh * LH : (lh + 1) * LH],
                            start=(ki == 0),
                            stop=(ki == KT_TILES - 1),
                        )
                    nc.scalar.activation(
                        gt_s[:, lh * LH : (lh + 1) * LH],
                        ps_g[:],
                        AF.Identity,
                        bias=bo_sb[:, j : j + 1],
                    )
                for a in range(KT_TILES // 4):
                    pt4 = ps_t_pool.tile([P, 4, P], F32R, name="pt4_out")
                    for b in range(4):
                        i = 4 * a + b
                        nc.tensor.transpose(
                            pt4[:, b, :], gt_s[:, i * P : (i + 1) * P], identr[:]
                        )
                    yb = yb_pool.tile([P, 4, P], F32, name="yb")
                    nc.vector.tensor_copy(yb[:], pt4[:])
                    nc.sync.dma_start(
                        y_d[4 * a * P : (4 * a + 4) * P, j * P : (j + 1) * P]
                        .rearrange("(b p) n -> p b n", p=P),
                        yb[:],
                    )

    nc.finalize()
    return nc


def _np_bf16():
    import ml_dtypes

    return ml_dtypes.bfloat16


def make_in_maps(inputs):
    """Host-side prep: rotate/shard x, cast to bf16, fold bv into bo."""
    bf16 = _np_bf16()
    q = np.asarray(inputs["q"], dtype=np.float32)
    Wq = np.asarray(inputs["Wq"], dtype=np.float32)
    Wk = np.asarray(inputs["Wk"], dtype=np.float32)
    Wv = np.asarray(inputs["Wv"], dtype=np.float32)
    Wo = np.asarray(inputs["Wo"], dtype=np.float32)
    bq = np.asarray(inputs["bq"], dtype=np.float32)
    bk = np.asarray(inputs["bk"], dtype=np.float32)
    bv = np.asarray(inputs["bv"], dtype=np.float32)
    bo = np.asarray(inputs["bo"], dtype=np.float32)

    bo_eff = (bv @ Wo + bo).astype(np.float32)
    x_dt = bf16 if TR_BF16 else np.float32
    wq_b = np.ascontiguousarray(Wq.astype(bf16))
    wk_b = np.ascontiguousarray(Wk.astype(bf16))
    wv_b = np.ascontiguousarray(Wv.astype(bf16))
    wo_b = np.ascontiguousarray(Wo.astype(bf16))

    in_maps = []
    for c in range(N_CORES):
        b, half = c // 2, c % 2
        lo = LQ * half
        x_rot = np.concatenate([q[b, lo:], q[b, :lo]], axis=0).astype(x_dt)
        in_maps.append({
            "x": np.ascontiguousarray(x_rot),
            "wq": wq_b, "wk": wk_b, "wv": wv_b, "wo": wo_b,
            "bq": bq, "bk": bk, "bo": bo_eff,
        })
    return in_maps


_NC_CACHE = None


def kernel(**inputs):
    global _NC_CACHE
    if _NC_CACHE is None:
        _NC_CACHE = build_nc()
    nc = _NC_CACHE

    in_maps = make_in_maps(inputs)
    res = run_bass_kernel_spmd(nc, in_maps, core_ids=list(range(N_CORES)))

    out = np.empty((B, L, D), dtype=np.float32)
    for c in range(N_CORES):
        b, half = c // 2, c % 2
        lo = LQ * half
        out[b, lo : lo + LQ, :] = np.asarray(res.results[c]["y"]).astype(np.float32)
    return out
